# revision 2
# baseline (speedup 1.0000x reference)
"""Trainium2 Bass kernel for the 2-layer LSTM LM problem.

Strategy: tensor-parallel over gate columns across 8 cores.
  - Each core owns a 128-wide slice of each gate (f,i,o,g) for both LSTM
    layers -> per-core weight slices [K, 512] stay SBUF-resident.
  - Step matmuls run with full batch B=128 as the PSUM partition dim and
    the 512 gate columns as the moving free dim (one PSUM bank).
  - After the gate math each core holds h_slice [128b, 128h]; a PE
    transpose + AllGather (8 cores) rebuilds full h^T [1024, 128] used as
    the stationary operand of the next step's matmuls.
  - The embedding is folded on the host: R0 = emb @ W0x_slice [256, 512],
    so the input-part contraction is over V=256 (host also pre-transposes
    inputs to [T, V, B]).
  - The output projection (logits_t = h1_t @ out_w) runs in-loop,
    replicated on every core; core 0's [T, B, V] result is returned.
"""

import numpy as np

import concourse.bass as bass
import concourse.mybir as mybir
import concourse.tile as tile
from concourse.bass_utils import run_bass_kernel_spmd
from concourse.masks import make_identity

T, B, V, E = 256, 128, 256, 512
N0 = N1 = 1024
N_CORES = 8
GS = 128          # per-core slice width of each gate
GW = 4 * GS       # 512 gate columns per core
FP = mybir.dt.float32


def split_excess_waits(nc, limit=1):
    """walrus in this env rejects >1 sem wait per instruction; spill excess
    on_wait entries onto same-engine Nops placed just before the owner."""
    import bass_rust

    for bb in nc.main_func.blocks:
        insts = bb.instructions
        i = 0
        while i < len(insts):
            ins = insts[i]
            si = getattr(ins, "sync_info", None)
            if si is None:
                i += 1
                continue
            waits = list(si.on_wait)
            if len(waits) <= limit:
                i += 1
                continue
            si.on_wait = waits[:limit]
            extra = waits[limit:]
            eng = ins.engine
            new_nops = []
            for s in range(0, len(extra), limit):
                chunk = extra[s : s + limit]
                nop = nc.engines[eng].nop(hint="waitsplit", nofuse=True).ins
                for b2 in nc.main_func.blocks:
                    if b2.instructions and b2.instructions[-1] is nop:
                        b2.instructions.pop()
                        break
                nop.sync_info = bass_rust.SyncInfo(on_wait=chunk, on_update=[])
                new_nops.append(nop)
            insts[i:i] = new_nops
            i += len(new_nops) + 1


def build_nc(t_steps=T):
    nc = bass.Bass("TRN2", target_bir_lowering=False, debug=False)

    inputsT = nc.dram_tensor("inputsT", [t_steps, V, B], FP, kind="ExternalInput")
    r0 = nc.dram_tensor("r0", [V, GW], FP, kind="ExternalInput")        # emb @ W0x slice
    w0h = nc.dram_tensor("w0h", [N0, GW], FP, kind="ExternalInput")
    w1x = nc.dram_tensor("w1x", [N0, GW], FP, kind="ExternalInput")
    w1h = nc.dram_tensor("w1h", [N1, GW], FP, kind="ExternalInput")
    outw = nc.dram_tensor("outw", [N1, V], FP, kind="ExternalInput")
    logits = nc.dram_tensor("logits", [t_steps, B, V], FP, kind="ExternalOutput")

    KC0 = N0 // 128   # h K-chunks (8)
    KCV = V // 128    # input K-chunks (2)

    with tile.TileContext(nc) as tc:
        with (
            tc.tile_pool(name="weights", bufs=1) as wpool,
            tc.tile_pool(name="state", bufs=1) as spool,
            tc.tile_pool(name="hT", bufs=3) as hpool,
            tc.tile_pool(name="work", bufs=3) as work,
            tc.tile_pool(name="inT", bufs=4) as inpool,
            tc.tile_pool(name="psg", bufs=2, space="PSUM") as psg,
            tc.tile_pool(name="pst", bufs=2, space="PSUM") as pst,
            tc.tile_pool(name="psl", bufs=2, space="PSUM") as psl_pool,
            tc.tile_pool(name="dram", bufs=4, space="DRAM") as dram,
        ):
            # ---- resident weights ----
            r0_sb = wpool.tile([128, KCV * GW], FP)        # [128, 2*512]
            for k in range(KCV):
                nc.sync.dma_start(r0_sb[:, k * GW:(k + 1) * GW], r0[k * 128:(k + 1) * 128, :])
            w0h_sb = wpool.tile([128, KC0 * GW], FP)       # [128, 8*512]
            w1x_sb = wpool.tile([128, KC0 * GW], FP)
            w1h_sb = wpool.tile([128, KC0 * GW], FP)
            for (wsb, wdr) in ((w0h_sb, w0h), (w1x_sb, w1x), (w1h_sb, w1h)):
                for k in range(KC0):
                    nc.sync.dma_start(wsb[:, k * GW:(k + 1) * GW], wdr[k * 128:(k + 1) * 128, :])
            outw_sb = wpool.tile([128, KC0 * V], FP)       # [128, 8*256]
            for k in range(KC0):
                nc.sync.dma_start(outw_sb[:, k * V:(k + 1) * V], outw[k * 128:(k + 1) * 128, :])
            ident = wpool.tile([128, 128], FP)
            make_identity(nc, ident[:])

            # ---- persistent state ----
            c0 = spool.tile([128, GS], FP)   # cell state slices [batch, hid_m]
            c1 = spool.tile([128, GS], FP)

            h0T_prev = None   # SBUF [128, 1024]: full h^T (chunk k at free 128k)
            h1T_prev = None

            for t in range(t_steps):
                # ---------------- layer 0 ----------------
                ps0 = psg.tile([128, GW], FP, tag="g0")
                # input part: lhsT = inputs_t^T chunks [v,128], rhs = R0 [v,512]
                for k in range(KCV):
                    itile = inpool.tile([128, 128], FP, tag="inT")
                    nc.sync.dma_start(itile[:], inputsT[t, k * 128:(k + 1) * 128, :])
                    nc.tensor.matmul(ps0[:], itile[:], r0_sb[:, k * GW:(k + 1) * GW],
                                     start=(k == 0),
                                     stop=(h0T_prev is None and k == KCV - 1))
                # recurrent part
                if h0T_prev is not None:
                    for k in range(KC0):
                        nc.tensor.matmul(ps0[:], h0T_prev[:, k * 128:(k + 1) * 128],
                                         w0h_sb[:, k * GW:(k + 1) * GW],
                                         start=False, stop=(k == KC0 - 1))

                h0T_full, c_new = _lstm_tail(nc, tc, work, pst, hpool, dram, psg,
                                             ps0, c0, ident, first=(t == 0), lid=0)

                # ---------------- layer 1 ----------------
                ps1 = psg.tile([128, GW], FP, tag="g1")
                if h1T_prev is not None:
                    for k in range(KC0):
                        nc.tensor.matmul(ps1[:], h1T_prev[:, k * 128:(k + 1) * 128],
                                         w1h_sb[:, k * GW:(k + 1) * GW],
                                         start=(k == 0), stop=False)
                for k in range(KC0):
                    nc.tensor.matmul(ps1[:], h0T_full[:, k * 128:(k + 1) * 128],
                                     w1x_sb[:, k * GW:(k + 1) * GW],
                                     start=(h1T_prev is None and k == 0),
                                     stop=(k == KC0 - 1))

                h1T_full, _ = _lstm_tail(nc, tc, work, pst, hpool, dram, psg,
                                         ps1, c1, ident, first=(t == 0), lid=1)

                # ---------------- output projection (replicated) ----------------
                psl = psl_pool.tile([128, V], FP, tag="logits")
                for k in range(KC0):
                    nc.tensor.matmul(psl[:], h1T_full[:, k * 128:(k + 1) * 128],
                                     outw_sb[:, k * V:(k + 1) * V],
                                     start=(k == 0), stop=(k == KC0 - 1))
                lsb = work.tile([128, V], FP, tag="lsb")
                nc.vector.tensor_copy(lsb[:], psl[:])
                nc.sync.dma_start(logits[t, :, :], lsb[:])

                h0T_prev, h1T_prev = h0T_full, h1T_full

    split_excess_waits(nc, limit=1)
    return nc


def _lstm_tail(nc, tc, work, pst, hpool, dram, psg, ps, c_state, ident, first, lid):
    """gate math + transpose + allgather; returns (hT_full [128,1024], None)."""
    FPL = FP
    # gate order in the 512 free cols: [f i o g]
    fio = work.tile([128, 3 * GS], FPL, tag=f"fio{lid}")
    g = work.tile([128, GS], FPL, tag=f"g{lid}")
    nc.scalar.activation(fio[:], ps[:, 0:3 * GS], mybir.ActivationFunctionType.Sigmoid)
    nc.scalar.activation(g[:], ps[:, 3 * GS:4 * GS], mybir.ActivationFunctionType.Tanh)

    if first:
        # c = i * g
        nc.vector.tensor_mul(c_state[:], fio[:, GS:2 * GS], g[:])
    else:
        t1 = work.tile([128, GS], FPL, tag=f"t1_{lid}")
        t2 = work.tile([128, GS], FPL, tag=f"t2_{lid}")
        nc.vector.tensor_mul(t1[:], fio[:, 0:GS], c_state[:])
        nc.vector.tensor_mul(t2[:], fio[:, GS:2 * GS], g[:])
        nc.vector.tensor_add(c_state[:], t1[:], t2[:])
    tc_t = work.tile([128, GS], FPL, tag=f"tc{lid}")
    nc.scalar.activation(tc_t[:], c_state[:], mybir.ActivationFunctionType.Tanh)
    h_sl = work.tile([128, GS], FPL, tag=f"h{lid}")
    nc.vector.tensor_mul(h_sl[:], fio[:, 2 * GS:3 * GS], tc_t[:])

    # transpose h slice -> [hid_m, batch]
    pt = pst.tile([128, 128], FPL, tag="tr")
    nc.tensor.transpose(pt[:], h_sl[:], ident[:])

    hT_sl = work.tile([128, 128], FPL, tag=f"hTs{lid}")
    nc.vector.tensor_copy(hT_sl[:], pt[:])

    # allgather across the 8 cores
    ag_in = dram.tile([128, 128], FPL, tag=f"agi{lid}")
    ag_out = dram.tile([N_CORES * 128, 128], FPL, tag=f"ago{lid}")
    nc.sync.dma_start(ag_in[:], hT_sl[:])
    nc.gpsimd.collective_compute(
        "AllGather",
        mybir.AluOpType.bypass,
        replica_groups=[list(range(N_CORES))],
        ins=[ag_in.opt()],
        outs=[ag_out.opt()],
    )
    hT = hpool.tile([128, N_CORES * 128], FPL, tag=f"hT{lid}")
    for r in range(N_CORES):
        nc.sync.dma_start(hT[:, r * 128:(r + 1) * 128], ag_out[r * 128:(r + 1) * 128, :])
    return hT, None


_NC_CACHE = {}


def _get_nc(t_steps):
    if t_steps not in _NC_CACHE:
        _NC_CACHE[t_steps] = build_nc(t_steps)
    return _NC_CACHE[t_steps]


def prep_in_maps(inputs, embedding_matrix, lstm_w0, lstm_w1, out_w, t_steps):
    inputs = np.asarray(inputs, np.float32)
    emb = np.asarray(embedding_matrix, np.float32)
    w0 = np.asarray(lstm_w0, np.float32)
    w1 = np.asarray(lstm_w1, np.float32)
    ow = np.ascontiguousarray(np.asarray(out_w, np.float32))

    inputsT = np.ascontiguousarray(inputs[:t_steps].transpose(0, 2, 1))  # [T, V, B]

    in_maps = []
    for m in range(N_CORES):
        cols = np.concatenate([np.arange(gi * 1024 + m * GS, gi * 1024 + (m + 1) * GS)
                               for gi in range(4)])
        w0s = np.ascontiguousarray(w0[:, cols])           # [1536, 512]
        w1s = np.ascontiguousarray(w1[:, cols])           # [2048, 512]
        r0 = np.ascontiguousarray(emb @ w0s[:E])          # [256, 512]
        in_maps.append({
            "inputsT": inputsT,
            "r0": r0,
            "w0h": np.ascontiguousarray(w0s[E:]),         # [1024, 512]
            "w1x": np.ascontiguousarray(w1s[:N0]),        # [1024, 512]
            "w1h": np.ascontiguousarray(w1s[N0:]),        # [1024, 512]
            "outw": ow,
        })
    return in_maps


LAST_EXEC_NS = None
LAST_TRACE = None


def kernel(inputs, embedding_matrix, lstm_w0, lstm_b0, lstm_w1, lstm_b1, out_w, out_b,
           _t_steps=None, _trace=False):
    global LAST_EXEC_NS, LAST_TRACE
    t_steps = _t_steps or inputs.shape[0]
    assert not np.any(lstm_b0) and not np.any(lstm_b1) and not np.any(out_b), \
        "nonzero biases not supported by this kernel build"

    nc = _get_nc(t_steps)
    in_maps = prep_in_maps(inputs, embedding_matrix, lstm_w0, lstm_w1, out_w, t_steps)

    res = run_bass_kernel_spmd(nc, in_maps, core_ids=list(range(N_CORES)),
                               trace=bool(_trace))
    if _trace:
        LAST_EXEC_NS = res.exec_time_ns
        LAST_TRACE = res.instructions_and_trace
    logits = res.results[0]["logits"]                     # [T, B, V]
    return np.ascontiguousarray(logits.reshape(t_steps * B, V))



# revision 5
# speedup vs baseline: 37.5156x; 37.5156x over previous
"""Trainium2 Bass kernel for the 2-layer LSTM LM problem.

Strategy: tensor-parallel over gate columns across 8 cores.
  - Each core owns a 128-wide slice of each gate (f,i,o,g) for both LSTM
    layers -> per-core weight slices [K, 512] stay SBUF-resident (bf16).
  - Step matmuls run with full batch B=128 as the PSUM partition dim and
    the 512 gate columns as the moving free dim (one PSUM bank).
  - All matmul operands are bf16 (1 cycle/row on the PE vs 4 for fp32);
    PSUM accumulation and the gate math stay fp32.
  - After the gate math each core holds h_slice [128b, 128h] (bf16); a
    PE transpose + AllGather (8 cores, bf16 payload) rebuilds full h^T
    [1024, 128] used as the stationary operand of later matmuls.
  - The loop is software-pipelined one layer deep: iteration i runs
    L0(step i), L1(step i-1), OUT(step i-2), so every AllGather gets a
    full iteration (~7us of other work) to land before its consumer.
  - The embedding is folded on the host: R0 = emb @ W0x_slice [256, 512],
    so the input-part contraction is over V=256 (host also pre-transposes
    inputs to [T, V, B] bf16).
  - The output projection (logits_t = h1_t @ out_w) is replicated on
    every core; core 0's [T, B, V] fp32 result is returned.
"""

import numpy as np
import ml_dtypes

import concourse.bass as bass
import concourse.mybir as mybir
import concourse.tile as tile
from concourse.bass_utils import run_bass_kernel_spmd
from concourse.masks import make_identity

T, B, V, E = 256, 128, 256, 512
N0 = N1 = 1024
N_CORES = 8
GS = 128          # per-core slice width of each gate
GW = 4 * GS       # 512 gate columns per core
FP = mybir.dt.float32
BF = mybir.dt.bfloat16
NP_BF = ml_dtypes.bfloat16
ACT = mybir.ActivationFunctionType


def split_excess_waits(nc, limit=1):
    """walrus in this env rejects >1 sem wait per instruction; spill excess
    on_wait entries onto same-engine Nops placed just before the owner."""
    import bass_rust

    for bb in nc.main_func.blocks:
        insts = bb.instructions
        i = 0
        while i < len(insts):
            ins = insts[i]
            si = getattr(ins, "sync_info", None)
            if si is None:
                i += 1
                continue
            waits = list(si.on_wait)
            if len(waits) <= limit:
                i += 1
                continue
            si.on_wait = waits[:limit]
            extra = waits[limit:]
            eng = ins.engine
            new_nops = []
            for s in range(0, len(extra), limit):
                chunk = extra[s : s + limit]
                nop = nc.engines[eng].nop(hint="waitsplit", nofuse=True).ins
                for b2 in nc.main_func.blocks:
                    if b2.instructions and b2.instructions[-1] is nop:
                        b2.instructions.pop()
                        break
                nop.sync_info = bass_rust.SyncInfo(on_wait=chunk, on_update=[])
                new_nops.append(nop)
            insts[i:i] = new_nops
            i += len(new_nops) + 1


def build_nc(t_steps=T):
    nc = bass.Bass("TRN2", target_bir_lowering=False, debug=False)

    inputsT = nc.dram_tensor("inputsT", [t_steps, V, B], BF, kind="ExternalInput")
    r0 = nc.dram_tensor("r0", [V, GW], BF, kind="ExternalInput")        # emb @ W0x slice
    w0h = nc.dram_tensor("w0h", [N0, GW], BF, kind="ExternalInput")
    w1x = nc.dram_tensor("w1x", [N0, GW], BF, kind="ExternalInput")
    w1h = nc.dram_tensor("w1h", [N1, GW], BF, kind="ExternalInput")
    outw = nc.dram_tensor("outw", [N1, V], BF, kind="ExternalInput")
    logits = nc.dram_tensor("logits", [t_steps, B, V], FP, kind="ExternalOutput")

    KC0 = N0 // 128   # h K-chunks (8)
    KCV = V // 128    # input K-chunks (2)

    with tile.TileContext(nc) as tc:
        with (
            tc.tile_pool(name="weights", bufs=1) as wpool,
            tc.tile_pool(name="state", bufs=1) as spool,
            tc.tile_pool(name="hT", bufs=3) as hpool,
            tc.tile_pool(name="work", bufs=3) as work,
            tc.tile_pool(name="inT", bufs=6) as inpool,
            tc.tile_pool(name="psg", bufs=2, space="PSUM") as psg,
            tc.tile_pool(name="pst", bufs=1, space="PSUM") as pst,
            tc.tile_pool(name="psl", bufs=2, space="PSUM") as psl_pool,
            tc.tile_pool(name="dram", bufs=4, space="DRAM") as dram,
        ):
            # ---- resident weights (bf16) ----
            r0_sb = wpool.tile([128, KCV * GW], BF)        # [128, 2*512]
            for k in range(KCV):
                nc.sync.dma_start(r0_sb[:, k * GW:(k + 1) * GW], r0[k * 128:(k + 1) * 128, :])
            w0h_sb = wpool.tile([128, KC0 * GW], BF)       # [128, 8*512]
            w1x_sb = wpool.tile([128, KC0 * GW], BF)
            w1h_sb = wpool.tile([128, KC0 * GW], BF)
            for (wsb, wdr) in ((w0h_sb, w0h), (w1x_sb, w1x), (w1h_sb, w1h)):
                for k in range(KC0):
                    nc.sync.dma_start(wsb[:, k * GW:(k + 1) * GW], wdr[k * 128:(k + 1) * 128, :])
            outw_sb = wpool.tile([128, KC0 * V], BF)       # [128, 8*256]
            for k in range(KC0):
                nc.sync.dma_start(outw_sb[:, k * V:(k + 1) * V], outw[k * 128:(k + 1) * 128, :])
            ident = wpool.tile([128, 128], BF)
            make_identity(nc, ident[:])

            # ---- persistent state ----
            c0 = spool.tile([128, GS], FP)   # cell state slices [batch, hid_m]
            c1 = spool.tile([128, GS], FP)

            h0T = [None] * t_steps   # SBUF bf16 [128, 1024]: full h^T per step
            h1T = [None] * t_steps

            def lstm_tail(ps, c_state, first, lid):
                """gate math + transpose + allgather; returns hT_full."""
                i_t = work.tile([128, GS], FP, tag=f"i{lid}")
                o_t = work.tile([128, GS], FP, tag=f"o{lid}")
                g_t = work.tile([128, GS], FP, tag=f"g{lid}")
                nc.scalar.activation(i_t[:], ps[:, GS:2 * GS], ACT.Sigmoid)
                nc.scalar.activation(g_t[:], ps[:, 3 * GS:4 * GS], ACT.Tanh)
                if first:
                    nc.vector.tensor_mul(c_state[:], i_t[:], g_t[:])
                else:
                    f_t = work.tile([128, GS], FP, tag=f"f{lid}")
                    t1 = work.tile([128, GS], FP, tag=f"t1_{lid}")
                    t2 = work.tile([128, GS], FP, tag=f"t2_{lid}")
                    nc.scalar.activation(f_t[:], ps[:, 0:GS], ACT.Sigmoid)
                    nc.vector.tensor_mul(t1[:], f_t[:], c_state[:])
                    nc.vector.tensor_mul(t2[:], i_t[:], g_t[:])
                    nc.vector.tensor_add(c_state[:], t1[:], t2[:])
                nc.scalar.activation(o_t[:], ps[:, 2 * GS:3 * GS], ACT.Sigmoid)
                tc_t = work.tile([128, GS], FP, tag=f"tc{lid}")
                nc.scalar.activation(tc_t[:], c_state[:], ACT.Tanh)
                h_sl = work.tile([128, GS], BF, tag=f"h{lid}")
                nc.vector.tensor_mul(h_sl[:], o_t[:], tc_t[:])

                # transpose h slice -> [hid_m, batch]
                pt = pst.tile([128, 128], BF, tag=f"tr{lid}")
                nc.tensor.transpose(pt[:], h_sl[:], ident[:])
                hT_sl = work.tile([128, 128], BF, tag=f"hTs{lid}")
                nc.scalar.copy(hT_sl[:], pt[:])

                # allgather across the 8 cores (bf16 payload)
                ag_in = dram.tile([128, 128], BF, tag=f"agi{lid}")
                ag_out = dram.tile([N_CORES * 128, 128], BF, tag=f"ago{lid}")
                nc.sync.dma_start(ag_in[:], hT_sl[:])
                nc.gpsimd.collective_compute(
                    "AllGather",
                    mybir.AluOpType.bypass,
                    replica_groups=[list(range(N_CORES))],
                    ins=[ag_in.opt()],
                    outs=[ag_out.opt()],
                )
                hT = hpool.tile([128, N_CORES * 128], BF, tag=f"hT{lid}")
                for r in range(N_CORES):
                    nc.sync.dma_start(hT[:, r * 128:(r + 1) * 128],
                                      ag_out[r * 128:(r + 1) * 128, :])
                return hT

            for i in range(t_steps + 2):
                # ---------------- layer 0, step i ----------------
                if i < t_steps:
                    ps0 = psg.tile([128, GW], FP, tag="g0")
                    for k in range(KCV):
                        itile = inpool.tile([128, 128], BF, tag="inT")
                        nc.sync.dma_start(itile[:], inputsT[i, k * 128:(k + 1) * 128, :])
                        nc.tensor.matmul(ps0[:], itile[:], r0_sb[:, k * GW:(k + 1) * GW],
                                         start=(k == 0),
                                         stop=(i == 0 and k == KCV - 1))
                    if i > 0:
                        hp = h0T[i - 1]
                        for k in range(KC0):
                            nc.tensor.matmul(ps0[:], hp[:, k * 128:(k + 1) * 128],
                                             w0h_sb[:, k * GW:(k + 1) * GW],
                                             start=False, stop=(k == KC0 - 1))
                    h0T[i] = lstm_tail(ps0, c0, first=(i == 0), lid=0)

                # ---------------- layer 1, step i-1 ----------------
                j = i - 1
                if 0 <= j < t_steps:
                    ps1 = psg.tile([128, GW], FP, tag="g1")
                    hx = h0T[j]
                    for k in range(KC0):
                        nc.tensor.matmul(ps1[:], hx[:, k * 128:(k + 1) * 128],
                                         w1x_sb[:, k * GW:(k + 1) * GW],
                                         start=(k == 0),
                                         stop=(j == 0 and k == KC0 - 1))
                    if j > 0:
                        hp = h1T[j - 1]
                        for k in range(KC0):
                            nc.tensor.matmul(ps1[:], hp[:, k * 128:(k + 1) * 128],
                                             w1h_sb[:, k * GW:(k + 1) * GW],
                                             start=False, stop=(k == KC0 - 1))
                    h1T[j] = lstm_tail(ps1, c1, first=(j == 0), lid=1)

                # ---------------- output projection, step i-2 ----------------
                o = i - 2
                if 0 <= o < t_steps:
                    psl = psl_pool.tile([128, V], FP, tag="logits")
                    ho = h1T[o]
                    for k in range(KC0):
                        nc.tensor.matmul(psl[:], ho[:, k * 128:(k + 1) * 128],
                                         outw_sb[:, k * V:(k + 1) * V],
                                         start=(k == 0), stop=(k == KC0 - 1))
                    lsb = work.tile([128, V], FP, tag="lsb")
                    nc.vector.tensor_copy(lsb[:], psl[:])
                    nc.sync.dma_start(logits[o, :, :], lsb[:])
                    h1T[o] = None   # release reference

    split_excess_waits(nc, limit=1)
    return nc


_NC_CACHE = {}


def _get_nc(t_steps):
    if t_steps not in _NC_CACHE:
        _NC_CACHE[t_steps] = build_nc(t_steps)
    return _NC_CACHE[t_steps]


def prep_in_maps(inputs, embedding_matrix, lstm_w0, lstm_w1, out_w, t_steps):
    inputs = np.asarray(inputs, np.float32)
    emb = np.asarray(embedding_matrix, np.float32)
    w0 = np.asarray(lstm_w0, np.float32)
    w1 = np.asarray(lstm_w1, np.float32)
    ow = np.asarray(out_w, np.float32)

    inputsT = np.ascontiguousarray(
        inputs[:t_steps].transpose(0, 2, 1)).astype(NP_BF)   # [T, V, B]

    in_maps = []
    for m in range(N_CORES):
        cols = np.concatenate([np.arange(gi * 1024 + m * GS, gi * 1024 + (m + 1) * GS)
                               for gi in range(4)])
        w0s = np.ascontiguousarray(w0[:, cols])           # [1536, 512]
        w1s = np.ascontiguousarray(w1[:, cols])           # [2048, 512]
        r0 = np.ascontiguousarray(emb @ w0s[:E])          # [256, 512]
        in_maps.append({
            "inputsT": inputsT,
            "r0": r0.astype(NP_BF),
            "w0h": np.ascontiguousarray(w0s[E:]).astype(NP_BF),    # [1024, 512]
            "w1x": np.ascontiguousarray(w1s[:N0]).astype(NP_BF),   # [1024, 512]
            "w1h": np.ascontiguousarray(w1s[N0:]).astype(NP_BF),   # [1024, 512]
            "outw": np.ascontiguousarray(ow).astype(NP_BF),
        })
    return in_maps


LAST_EXEC_NS = None
LAST_TRACE = None


def kernel(inputs, embedding_matrix, lstm_w0, lstm_b0, lstm_w1, lstm_b1, out_w, out_b,
           _t_steps=None, _trace=False):
    global LAST_EXEC_NS, LAST_TRACE
    t_steps = _t_steps or inputs.shape[0]
    assert not np.any(lstm_b0) and not np.any(lstm_b1) and not np.any(out_b), \
        "nonzero biases not supported by this kernel build"

    nc = _get_nc(t_steps)
    in_maps = prep_in_maps(inputs, embedding_matrix, lstm_w0, lstm_w1, out_w, t_steps)

    res = run_bass_kernel_spmd(nc, in_maps, core_ids=list(range(N_CORES)),
                               trace=bool(_trace))
    if _trace:
        LAST_EXEC_NS = res.exec_time_ns
        LAST_TRACE = res.instructions_and_trace
    logits = res.results[0]["logits"]                     # [T, B, V]
    return np.ascontiguousarray(logits.reshape(t_steps * B, V))


# revision 11
# speedup vs baseline: 43.1129x; 1.1492x over previous
"""Trainium2 Bass kernel for the 2-layer LSTM LM problem.

Strategy: tensor-parallel over gate columns across 8 cores.
  - Each core owns a 128-wide slice of each gate (f,i,o,g) for both LSTM
    layers -> per-core weight slices [K, 512] stay SBUF-resident (bf16).
  - Step matmuls run with full batch B=128 as the PSUM partition dim and
    the 512 gate columns as the moving free dim (one PSUM bank).
  - All matmul operands are bf16 (1 cycle/row on the PE vs 4 for fp32);
    PSUM accumulation and the gate math stay fp32.
  - After the gate math each core holds h_slice [128b, 128h] (bf16); a
    PE transpose + AllGather (8 cores, bf16 payload) rebuilds full h^T
    [1024, 128] used as the stationary operand of later matmuls.
  - The loop is software-pipelined one layer deep: iteration i runs
    L0(step i), L1(step i-1), OUT(step i-2), so every AllGather gets a
    full iteration (~7us of other work) to land before its consumer.
  - The embedding is folded on the host: R0 = emb @ W0x_slice [256, 512],
    so the input-part contraction is over V=256 (host also pre-transposes
    inputs to [T, V, B] bf16).
  - The output projection (logits_t = h1_t @ out_w) is replicated on
    every core; core 0's [T, B, V] fp32 result is returned.
"""

import numpy as np
import ml_dtypes

import concourse.bass as bass
import concourse.mybir as mybir
import concourse.tile as tile
from concourse.bass_utils import run_bass_kernel_spmd
from concourse.masks import make_identity

T, B, V, E = 256, 128, 256, 512
N0 = N1 = 1024
N_CORES = 8
GS = 128          # per-core slice width of each gate
GW = 4 * GS       # 512 gate columns per core
FP = mybir.dt.float32
BF = mybir.dt.bfloat16
NP_BF = ml_dtypes.bfloat16
ACT = mybir.ActivationFunctionType


def split_excess_waits(nc, limit=1):
    """walrus in this env rejects >1 sem wait per instruction; spill excess
    on_wait entries onto same-engine Nops placed just before the owner."""
    import bass_rust

    for bb in nc.main_func.blocks:
        insts = bb.instructions
        i = 0
        while i < len(insts):
            ins = insts[i]
            si = getattr(ins, "sync_info", None)
            if si is None:
                i += 1
                continue
            waits = list(si.on_wait)
            if len(waits) <= limit:
                i += 1
                continue
            si.on_wait = waits[:limit]
            extra = waits[limit:]
            eng = ins.engine
            new_nops = []
            for s in range(0, len(extra), limit):
                chunk = extra[s : s + limit]
                nop = nc.engines[eng].nop(hint="waitsplit", nofuse=True).ins
                for b2 in nc.main_func.blocks:
                    if b2.instructions and b2.instructions[-1] is nop:
                        b2.instructions.pop()
                        break
                nop.sync_info = bass_rust.SyncInfo(on_wait=chunk, on_update=[])
                new_nops.append(nop)
            insts[i:i] = new_nops
            i += len(new_nops) + 1


def build_nc(t_steps=T):
    nc = bass.Bass("TRN2", target_bir_lowering=False, debug=False)

    inputsT = nc.dram_tensor("inputsT", [t_steps, V, B], BF, kind="ExternalInput")
    r0 = nc.dram_tensor("r0", [V, GW], BF, kind="ExternalInput")        # emb @ W0x slice
    w0h = nc.dram_tensor("w0h", [N0, GW], BF, kind="ExternalInput")
    w1x = nc.dram_tensor("w1x", [N0, GW], BF, kind="ExternalInput")
    w1h = nc.dram_tensor("w1h", [N1, GW], BF, kind="ExternalInput")
    outw = nc.dram_tensor("outw", [N1, V], BF, kind="ExternalInput")
    logits = nc.dram_tensor("logits", [t_steps, B, V], FP, kind="ExternalOutput")

    KC0 = N0 // 128   # h K-chunks (8)
    KCV = V // 128    # input K-chunks (2)

    with tile.TileContext(nc) as tc:
        with (
            tc.tile_pool(name="weights", bufs=1) as wpool,
            tc.tile_pool(name="state", bufs=1) as spool,
            tc.tile_pool(name="hT", bufs=3) as hpool,
            tc.tile_pool(name="work", bufs=3) as work,
            tc.tile_pool(name="inT", bufs=6) as inpool,
            tc.tile_pool(name="psg", bufs=2, space="PSUM") as psg,
            tc.tile_pool(name="pst", bufs=1, space="PSUM") as pst,
            tc.tile_pool(name="psl", bufs=2, space="PSUM") as psl_pool,
            tc.tile_pool(name="dram", bufs=4, space="DRAM") as dram,
        ):
            # ---- resident weights (bf16) ----
            r0_sb = wpool.tile([128, KCV * GW], BF)        # [128, 2*512]
            for k in range(KCV):
                nc.sync.dma_start(r0_sb[:, k * GW:(k + 1) * GW], r0[k * 128:(k + 1) * 128, :])
            w0h_sb = wpool.tile([128, KC0 * GW], BF)       # [128, 8*512]
            w1x_sb = wpool.tile([128, KC0 * GW], BF)
            w1h_sb = wpool.tile([128, KC0 * GW], BF)
            for (wsb, wdr) in ((w0h_sb, w0h), (w1x_sb, w1x), (w1h_sb, w1h)):
                for k in range(KC0):
                    nc.sync.dma_start(wsb[:, k * GW:(k + 1) * GW], wdr[k * 128:(k + 1) * 128, :])
            outw_sb = wpool.tile([128, KC0 * V], BF)       # [128, 8*256]
            for k in range(KC0):
                nc.sync.dma_start(outw_sb[:, k * V:(k + 1) * V], outw[k * 128:(k + 1) * 128, :])
            ident = wpool.tile([128, 128], BF)
            make_identity(nc, ident[:])

            # ---- persistent state ----
            c0 = spool.tile([128, GS], FP)   # cell state slices [batch, hid_m]
            c1 = spool.tile([128, GS], FP)

            h0T = [None] * t_steps   # SBUF bf16 [128, 1024]: full h^T per step
            h1T = [None] * t_steps

            def lstm_tail(ps, c_state, first, lid):
                """gate math + transpose + allgather issue; returns ag_out."""
                i_t = work.tile([128, GS], FP, tag=f"i{lid}")
                o_t = work.tile([128, GS], FP, tag=f"o{lid}")
                g_t = work.tile([128, GS], FP, tag=f"g{lid}")
                nc.scalar.activation(i_t[:], ps[:, GS:2 * GS], ACT.Sigmoid)
                nc.scalar.activation(g_t[:], ps[:, 3 * GS:4 * GS], ACT.Tanh)
                if first:
                    nc.vector.tensor_mul(c_state[:], i_t[:], g_t[:])
                else:
                    f_t = work.tile([128, GS], FP, tag=f"f{lid}")
                    t1 = work.tile([128, GS], FP, tag=f"t1_{lid}")
                    t2 = work.tile([128, GS], FP, tag=f"t2_{lid}")
                    nc.scalar.activation(f_t[:], ps[:, 0:GS], ACT.Sigmoid)
                    nc.vector.tensor_mul(t1[:], f_t[:], c_state[:])
                    nc.vector.tensor_mul(t2[:], i_t[:], g_t[:])
                    nc.vector.tensor_add(c_state[:], t1[:], t2[:])
                nc.scalar.activation(o_t[:], ps[:, 2 * GS:3 * GS], ACT.Sigmoid)
                tc_t = work.tile([128, GS], FP, tag=f"tc{lid}")
                nc.scalar.activation(tc_t[:], c_state[:], ACT.Tanh)
                h_sl = work.tile([128, GS], BF, tag=f"h{lid}")
                nc.vector.tensor_mul(h_sl[:], o_t[:], tc_t[:])

                # transpose h slice -> [hid_m, batch]
                pt = pst.tile([128, 128], BF, tag=f"tr{lid}")
                nc.tensor.transpose(pt[:], h_sl[:], ident[:])
                hT_sl = work.tile([128, 128], BF, tag=f"hTs{lid}")
                nc.scalar.copy(hT_sl[:], pt[:])

                # allgather across the 8 cores (bf16 payload); input bounce
                # rides the scalar queue (right after the copy above), the
                # trigger rides gpsimd.  The gather-back to SBUF is deferred
                # to the consumer (gather_back below).
                ag_in = dram.tile([128, 128], BF, tag=f"agi{lid}")
                ag_out = dram.tile([N_CORES * 128, 128], BF, tag=f"ago{lid}")
                nc.scalar.dma_start(ag_in[:], hT_sl[:])
                nc.gpsimd.collective_compute(
                    "AllGather",
                    mybir.AluOpType.bypass,
                    replica_groups=[list(range(N_CORES))],
                    ins=[ag_in.opt()],
                    outs=[ag_out.opt()],
                )
                return ag_out

            def gather_back(ag_out, lid):
                """single strided DMA DRAM->SBUF rebuilding hT [128, 1024].

                Dedicated to the sync queue: the only instructions on sync
                are these gather DMAs, ordered by collective completion, so
                a wait here never blocks an unrelated AG's input bounce.
                """
                hT = hpool.tile([128, N_CORES * 128], BF, tag=f"hT{lid}")
                src = ag_out[:].rearrange("(r p) b -> p r b", r=N_CORES)
                nc.sync.dma_start(hT[:], src)
                return hT

            ag0 = [None] * t_steps   # pending allgather outputs (DRAM)
            ag1 = [None] * t_steps
            h0T = [None] * t_steps   # gathered h^T tiles (SBUF)
            h1T = [None] * t_steps

            for i in range(t_steps + 2):
                # ---------------- layer 0, step i ----------------
                if 1 <= i <= t_steps:
                    h0T[i - 1] = gather_back(ag0[i - 1], 0)
                if i < t_steps:
                    ps0 = psg.tile([128, GW], FP, tag="g0")
                    itile = inpool.tile([128, KCV * 128], BF, tag="inT")
                    nc.gpsimd.dma_start(
                        itile[:],
                        inputsT[i].rearrange("(k p) b -> p k b", k=KCV))
                    for k in range(KCV):
                        nc.tensor.matmul(ps0[:], itile[:, k * 128:(k + 1) * 128],
                                         r0_sb[:, k * GW:(k + 1) * GW],
                                         start=(k == 0),
                                         stop=(i == 0 and k == KCV - 1))
                    if i > 0:
                        hp = h0T[i - 1]
                        for k in range(KC0):
                            nc.tensor.matmul(ps0[:], hp[:, k * 128:(k + 1) * 128],
                                             w0h_sb[:, k * GW:(k + 1) * GW],
                                             start=False, stop=(k == KC0 - 1))
                    ag0[i] = lstm_tail(ps0, c0, first=(i == 0), lid=0)

                # ---------------- layer 1, step i-1 ----------------
                j = i - 1
                if 1 <= j <= t_steps:
                    h1T[j - 1] = gather_back(ag1[j - 1], 1)
                if 0 <= j < t_steps:
                    ps1 = psg.tile([128, GW], FP, tag="g1")
                    hx = h0T[j]
                    for k in range(KC0):
                        nc.tensor.matmul(ps1[:], hx[:, k * 128:(k + 1) * 128],
                                         w1x_sb[:, k * GW:(k + 1) * GW],
                                         start=(k == 0),
                                         stop=(j == 0 and k == KC0 - 1))
                    if j > 0:
                        hp = h1T[j - 1]
                        for k in range(KC0):
                            nc.tensor.matmul(ps1[:], hp[:, k * 128:(k + 1) * 128],
                                             w1h_sb[:, k * GW:(k + 1) * GW],
                                             start=False, stop=(k == KC0 - 1))
                    ag1[j] = lstm_tail(ps1, c1, first=(j == 0), lid=1)

                # ---------------- output projection, step i-2 ----------------
                o = i - 2
                if 0 <= o < t_steps:
                    psl = psl_pool.tile([128, V], FP, tag="logits")
                    ho = h1T[o]
                    for k in range(KC0):
                        nc.tensor.matmul(psl[:], ho[:, k * 128:(k + 1) * 128],
                                         outw_sb[:, k * V:(k + 1) * V],
                                         start=(k == 0), stop=(k == KC0 - 1))
                    lsb = work.tile([128, V], FP, tag="lsb")
                    nc.vector.tensor_copy(lsb[:], psl[:])
                    nc.gpsimd.dma_start(logits[o, :, :], lsb[:])
                    h1T[o] = None
                    ag1[o] = None
                    if o > 0:
                        h0T[o - 1] = None
                        ag0[o - 1] = None

    split_excess_waits(nc, limit=1)
    return nc


_NC_CACHE = {}


def _get_nc(t_steps):
    if t_steps not in _NC_CACHE:
        _NC_CACHE[t_steps] = build_nc(t_steps)
    return _NC_CACHE[t_steps]


def prep_in_maps(inputs, embedding_matrix, lstm_w0, lstm_w1, out_w, t_steps):
    inputs = np.asarray(inputs, np.float32)
    emb = np.asarray(embedding_matrix, np.float32)
    w0 = np.asarray(lstm_w0, np.float32)
    w1 = np.asarray(lstm_w1, np.float32)
    ow = np.asarray(out_w, np.float32)

    inputsT = np.ascontiguousarray(
        inputs[:t_steps].transpose(0, 2, 1)).astype(NP_BF)   # [T, V, B]

    in_maps = []
    for m in range(N_CORES):
        cols = np.concatenate([np.arange(gi * 1024 + m * GS, gi * 1024 + (m + 1) * GS)
                               for gi in range(4)])
        w0s = np.ascontiguousarray(w0[:, cols])           # [1536, 512]
        w1s = np.ascontiguousarray(w1[:, cols])           # [2048, 512]
        r0 = np.ascontiguousarray(emb @ w0s[:E])          # [256, 512]
        in_maps.append({
            "inputsT": inputsT,
            "r0": r0.astype(NP_BF),
            "w0h": np.ascontiguousarray(w0s[E:]).astype(NP_BF),    # [1024, 512]
            "w1x": np.ascontiguousarray(w1s[:N0]).astype(NP_BF),   # [1024, 512]
            "w1h": np.ascontiguousarray(w1s[N0:]).astype(NP_BF),   # [1024, 512]
            "outw": np.ascontiguousarray(ow).astype(NP_BF),
        })
    return in_maps


LAST_EXEC_NS = None
LAST_TRACE = None


def kernel(inputs, embedding_matrix, lstm_w0, lstm_b0, lstm_w1, lstm_b1, out_w, out_b,
           _t_steps=None, _trace=False):
    global LAST_EXEC_NS, LAST_TRACE
    t_steps = _t_steps or inputs.shape[0]
    assert not np.any(lstm_b0) and not np.any(lstm_b1) and not np.any(out_b), \
        "nonzero biases not supported by this kernel build"

    nc = _get_nc(t_steps)
    in_maps = prep_in_maps(inputs, embedding_matrix, lstm_w0, lstm_w1, out_w, t_steps)

    res = run_bass_kernel_spmd(nc, in_maps, core_ids=list(range(N_CORES)),
                               trace=bool(_trace))
    if _trace:
        LAST_EXEC_NS = res.exec_time_ns
        LAST_TRACE = res.instructions_and_trace
    logits = res.results[0]["logits"]                     # [T, B, V]
    return np.ascontiguousarray(logits.reshape(t_steps * B, V))
